# revision 18
# baseline (speedup 1.0000x reference)
"""Bass/Trainium2 kernel for nn_AdvancedUpSampling2D (max-unpooling via scatter).

Full tensors in/out; internally sharded batch-parallel over 8 NeuronCores.

Key structural fact about the mask (argmax-style, include_batch_in_index=False):
  flat = (y * Wout + x) * C + c  with y = 2h + dy, x = 2w + dx, dy/dx in {0,1}
  Wout * C = 128 * 256 = 2^15, C = 2^8
  => dy = bit 15 of flat, dx = bit 8 of flat, and element (b,h,w,c) can only
     land at (b, 2h+dy, 2w+dx, c).  Windows are disjoint => no add-collisions.
So the scatter is a 4-way select + spatial interleave:
  out[b, 2h+dy', 2w+dx', c] = updates[b,h,w,c] * ((mask & 0x8100) == K(dy',dx'))

Engine split per chunk:
  DVE:     s = m & 0x8100 (f32 out), 4x tensor_tensor selects into the
           interleaved row tiles (the 1x-mode fp32 floor).
  ScalarE: per-plane indicator = Relu(1 - Square(s - K)) (2 ACT ops/plane),
           plus kicks for half the DMA traffic (Act HWDGE queue).
  SP:      kicks for the other half of DMA traffic (SP HWDGE queue).
"""

import numpy as np

# Problem config (hardcoded per contract)
B, H, W, C = 16, 64, 64, 256
SY, SX = 2, 2
N_CORES = 8
BPC = B // N_CORES          # batches per core = 2
P = 128                     # partitions = BPC * H
CW = 8                      # W-chunk per tile
NCHUNK = W // CW            # 8 chunks

_CACHE = {}


def _build_module():
    """Build the Bass module (single-core program, run SPMD on 8 cores)."""
    import concourse.bacc as bacc
    import concourse.tile as tile
    from concourse import mybir

    nc = bacc.Bacc(
        "TRN2",
        target_bir_lowering=False,
        debug=False,
        num_devices=N_CORES,
    )
    upd = nc.dram_tensor(
        "updates", [BPC, H, W, C], mybir.dt.float32, kind="ExternalInput"
    )
    msk = nc.dram_tensor("mask", [BPC, H, W, C], mybir.dt.int32, kind="ExternalInput")
    out = nc.dram_tensor(
        "out", [BPC, H * SY, W * SX, C], mybir.dt.float32, kind="ExternalOutput"
    )

    up_ap = upd.ap()                      # [2, 64, 64, 256]
    mk_ap = msk.ap()
    # out rows r = 2h + two  ->  [b, two, h, q, c]
    out_r = out.ap().rearrange("b (h two) q c -> b two h q c", two=SY)

    # (plane key, which row-parity tile, slot within the (w, two, c) interleave)
    PLANES = [
        (0x0000, 0, 0),  # dy=0, dx=0 -> even row, even col
        (0x0100, 0, 1),  # dy=0, dx=1 -> even row, odd col
        (0x8000, 1, 0),  # dy=1, dx=0 -> odd row, even col
        (0x8100, 1, 1),  # dy=1, dx=1 -> odd row, odd col
    ]

    with tile.TileContext(nc) as tc:
        with (
            tc.tile_pool(name="bias", bufs=1) as bias_pool,
            tc.tile_pool(name="in", bufs=4) as in_pool,
            tc.tile_pool(name="s", bufs=2) as s_pool,
            tc.tile_pool(name="eq", bufs=6) as eq_pool,
            tc.tile_pool(name="out", bufs=2) as out_pool,
        ):
            # Sigmoid step biases as dep-tracked tiles (avoids a preamble
            # all-engine barrier for const-AP registration).
            bias_lo = bias_pool.tile([P, 1], mybir.dt.float32, name="bias_lo")
            bias_hi = bias_pool.tile([P, 1], mybir.dt.float32, name="bias_hi")
            nc.gpsimd.memset(bias_lo[:], 128.0)
            nc.gpsimd.memset(bias_hi[:], -32896.0)
            for j in range(NCHUNK):
                w0 = j * CW
                u_t = in_pool.tile([P, CW * C], mybir.dt.float32)
                m_t = in_pool.tile([P, CW * C], mybir.dt.int32)
                # One dma_start per tensor covering both batch rows: SBUF
                # partition p = b*64 + h walks [b, h] in the same order as
                # the 4D DRAM AP, and balance_dma_aps matches total sizes.
                nc.scalar.dma_start(
                    out=u_t[:].rearrange("p (w c) -> p w c", c=C),
                    in_=up_ap[:, :, w0 : w0 + CW, :].rearrange(
                        "b h w c -> (b h) w c"
                    ),
                )
                nc.sync.dma_start(
                    out=m_t[:].rearrange("p (w c) -> p w c", c=C),
                    in_=mk_ap[:, :, w0 : w0 + CW, :].rearrange("b h w c -> (b h) w c"),
                )

                even_t = out_pool.tile([P, SX * CW * C], mybir.dt.float32)
                odd_t = out_pool.tile([P, SX * CW * C], mybir.dt.float32)
                row_tiles = [even_t, odd_t]

                u_v = u_t[:].rearrange("p (w c) -> p w c", c=C)
                # s = m & 0x8100 (values in {0,256,32768,33024}); ACT reads
                # int32 directly (converts on input).
                s_t = s_pool.tile([P, CW * C], mybir.dt.int32)
                nc.vector.tensor_scalar(
                    out=s_t[:],
                    in0=m_t[:],
                    scalar1=0x8100,
                    scalar2=None,
                    op0=mybir.AluOpType.bitwise_and,
                )
                for key, parity, slot in PLANES:
                    eq = eq_pool.tile([P, CW * C], mybir.dt.float32, name="eq")
                    if key == 0x0000:
                        # s==0 <=> s<128: saturated step, one ScalarE op.
                        # |arg| >= 128 always, so sigmoid returns exact 0/1.
                        nc.scalar.activation(
                            eq[:],
                            s_t[:],
                            mybir.ActivationFunctionType.Sigmoid,
                            bias=bias_lo[:],
                            scale=-1.0,
                        )
                    elif key == 0x8100:
                        # s==33024 <=> s>32896: saturated step, one ScalarE op
                        nc.scalar.activation(
                            eq[:],
                            s_t[:],
                            mybir.ActivationFunctionType.Sigmoid,
                            bias=bias_hi[:],
                            scale=1.0,
                        )
                    else:
                        # middle keys: exact is_equal on DVE (2x-mode TS)
                        nc.vector.tensor_scalar(
                            out=eq[:],
                            in0=s_t[:],
                            scalar1=key,
                            scalar2=None,
                            op0=mybir.AluOpType.is_equal,
                        )
                    dst = row_tiles[parity][:].rearrange(
                        "p (w two c) -> p w two c", two=SX, c=C
                    )[:, :, slot, :]
                    nc.vector.tensor_mul(
                        out=dst,
                        in0=u_v,
                        in1=eq[:].rearrange("p (w c) -> p w c", c=C),
                    )

                for parity in range(SY):
                    # even rows store via Act HWDGE, odd rows via SP HWDGE
                    dma_eng = nc.scalar if parity == 0 else nc.sync
                    dma_eng.dma_start(
                        out=out_r[:, parity, :, SX * w0 : SX * (w0 + CW), :].rearrange(
                            "b h q c -> (b h) q c"
                        ),
                        in_=row_tiles[parity][:].rearrange("p (q c) -> p q c", c=C),
                    )
    nc.finalize()
    return nc


def _run(updates, mask, trace=False):
    from concourse.bass_utils import run_bass_kernel_spmd

    if "nc" not in _CACHE:
        _CACHE["nc"] = _build_module()
    nc = _CACHE["nc"]

    updates = np.ascontiguousarray(updates, dtype=np.float32)
    mask = np.ascontiguousarray(mask, dtype=np.int32)
    in_maps = [
        {
            "updates": updates[i * BPC : (i + 1) * BPC],
            "mask": mask[i * BPC : (i + 1) * BPC],
        }
        for i in range(N_CORES)
    ]
    res = run_bass_kernel_spmd(
        nc,
        in_maps,
        core_ids=list(range(N_CORES)),
        trace=trace,
    )
    out = np.concatenate([r["out"] for r in res.results], axis=0)
    return out, res


def kernel(**inputs):
    out, _ = _run(inputs["updates"], inputs["mask"])
    return out


# revision 23
# speedup vs baseline: 1.1962x; 1.1962x over previous
"""Bass/Trainium2 kernel for nn_AdvancedUpSampling2D (max-unpooling via scatter).

Full tensors in/out; internally sharded batch-parallel over 8 NeuronCores.

Key structural fact about the mask (argmax-style, include_batch_in_index=False):
  flat = (y * Wout + x) * C + c  with y = 2h + dy, x = 2w + dx, dy/dx in {0,1}
  Wout * C = 128 * 256 = 2^15, C = 2^8
  => dy = bit 15 of flat, dx = bit 8 of flat, and element (b,h,w,c) can only
     land at (b, 2h+dy, 2w+dx, c).  Windows are disjoint => no add-collisions.
So the scatter is a 4-way select + spatial interleave:
  out[b, 2h+dy', 2w+dx', c] = updates[b,h,w,c] * ((mask & 0x8100) == K(dy',dx'))

Engine split per chunk:
  DVE:     s = m & 0x8100 (int32), is_equal indicators for the two middle
           keys, and 4x tensor_tensor selects into the interleaved row
           tiles (the 1x-mode fp32 floor, ~73us/core).
  ScalarE: indicators for the extreme keys (0x0000/0x8100) as saturated
           Sigmoid steps (one ACT op each; |arg| >= 128 so the result is
           exactly 0.0/1.0), plus kicks for half the DMA traffic (Act
           HWDGE queue: updates loads + even-row stores).
  SP:      kicks for the other half of DMA traffic (SP HWDGE queue:
           mask loads + odd-row stores).
Two HWDGE queues together sustain ~420 GB/s, the per-core share of the
HBM domain; the kernel is memory-roofline-bound at ~131 us/core.
"""

import numpy as np

# Problem config (hardcoded per contract)
B, H, W, C = 16, 64, 64, 256
SY, SX = 2, 2
N_CORES = 8
BPC = B // N_CORES          # batches per core = 2
P = 128                     # partitions = BPC * H
CW = 8                      # W-chunk per tile
NCHUNK = W // CW            # 8 chunks

_CACHE = {}


def _build_module():
    """Build the Bass module (single-core program, run SPMD on 8 cores)."""
    import concourse.bacc as bacc
    import concourse.tile as tile
    from concourse import mybir

    nc = bacc.Bacc(
        "TRN2",
        target_bir_lowering=False,
        debug=False,
        num_devices=N_CORES,
    )
    # Bias constants for the ScalarE activations (only 0.0/1.0 pre-registered).
    for v in (128.0, -32896.0):
        t = nc.alloc_sbuf_tensor(f"const-float32-{v}", [128, 1], mybir.dt.float32)
        nc.gpsimd.memset(t.ap(), v)
        nc.const_aps.aps[(mybir.dt.float32, v)] = t.ap()
    nc.all_engine_barrier()

    upd = nc.dram_tensor(
        "updates", [BPC, H, W, C], mybir.dt.float32, kind="ExternalInput"
    )
    msk = nc.dram_tensor("mask", [BPC, H, W, C], mybir.dt.int32, kind="ExternalInput")
    out = nc.dram_tensor(
        "out", [BPC, H * SY, W * SX, C], mybir.dt.float32, kind="ExternalOutput"
    )

    up_ap = upd.ap()                      # [2, 64, 64, 256]
    mk_ap = msk.ap()
    # out rows r = 2h + two  ->  [b, two, h, q, c]
    out_r = out.ap().rearrange("b (h two) q c -> b two h q c", two=SY)

    # (plane key, which row-parity tile, slot within the (w, two, c) interleave)
    PLANES = [
        (0x0000, 0, 0),  # dy=0, dx=0 -> even row, even col
        (0x0100, 0, 1),  # dy=0, dx=1 -> even row, odd col
        (0x8000, 1, 0),  # dy=1, dx=0 -> odd row, even col
        (0x8100, 1, 1),  # dy=1, dx=1 -> odd row, odd col
    ]

    with tile.TileContext(nc) as tc:
        with (
            tc.tile_pool(name="in", bufs=4) as in_pool,
            tc.tile_pool(name="s", bufs=2) as s_pool,
            tc.tile_pool(name="eq", bufs=6) as eq_pool,
            tc.tile_pool(name="out", bufs=2) as out_pool,
        ):
            for j in range(NCHUNK):
                w0 = j * CW
                u_t = in_pool.tile([P, CW * C], mybir.dt.float32)
                m_t = in_pool.tile([P, CW * C], mybir.dt.int32)
                # One dma_start per tensor covering both batch rows: SBUF
                # partition p = b*64 + h walks [b, h] in the same order as
                # the 4D DRAM AP, and balance_dma_aps matches total sizes.
                nc.scalar.dma_start(
                    out=u_t[:].rearrange("p (w c) -> p w c", c=C),
                    in_=up_ap[:, :, w0 : w0 + CW, :].rearrange(
                        "b h w c -> (b h) w c"
                    ),
                )
                nc.sync.dma_start(
                    out=m_t[:].rearrange("p (w c) -> p w c", c=C),
                    in_=mk_ap[:, :, w0 : w0 + CW, :].rearrange("b h w c -> (b h) w c"),
                )

                even_t = out_pool.tile([P, SX * CW * C], mybir.dt.float32)
                odd_t = out_pool.tile([P, SX * CW * C], mybir.dt.float32)
                row_tiles = [even_t, odd_t]

                u_v = u_t[:].rearrange("p (w c) -> p w c", c=C)
                # s = m & 0x8100 (values in {0,256,32768,33024}); ACT reads
                # int32 directly (converts on input).
                s_t = s_pool.tile([P, CW * C], mybir.dt.int32)
                nc.vector.tensor_scalar(
                    out=s_t[:],
                    in0=m_t[:],
                    scalar1=0x8100,
                    scalar2=None,
                    op0=mybir.AluOpType.bitwise_and,
                )
                for key, parity, slot in PLANES:
                    eq = eq_pool.tile([P, CW * C], mybir.dt.float32, name="eq")
                    if key == 0x0000:
                        # s==0 <=> s<128: saturated step, one ScalarE op.
                        # |arg| >= 128 always, so sigmoid returns exact 0/1.
                        nc.scalar.activation(
                            eq[:],
                            s_t[:],
                            mybir.ActivationFunctionType.Sigmoid,
                            bias=128.0,
                            scale=-1.0,
                        )
                    elif key == 0x8100:
                        # s==33024 <=> s>32896: saturated step, one ScalarE op
                        nc.scalar.activation(
                            eq[:],
                            s_t[:],
                            mybir.ActivationFunctionType.Sigmoid,
                            bias=-32896.0,
                            scale=1.0,
                        )
                    else:
                        # middle keys: exact is_equal on DVE (2x-mode TS)
                        nc.vector.tensor_scalar(
                            out=eq[:],
                            in0=s_t[:],
                            scalar1=key,
                            scalar2=None,
                            op0=mybir.AluOpType.is_equal,
                        )
                    dst = row_tiles[parity][:].rearrange(
                        "p (w two c) -> p w two c", two=SX, c=C
                    )[:, :, slot, :]
                    nc.vector.tensor_mul(
                        out=dst,
                        in0=u_v,
                        in1=eq[:].rearrange("p (w c) -> p w c", c=C),
                    )

                for parity in range(SY):
                    # even rows store via Act HWDGE, odd rows via SP HWDGE
                    dma_eng = nc.scalar if parity == 0 else nc.sync
                    dma_eng.dma_start(
                        out=out_r[:, parity, :, SX * w0 : SX * (w0 + CW), :].rearrange(
                            "b h q c -> (b h) q c"
                        ),
                        in_=row_tiles[parity][:].rearrange("p (q c) -> p q c", c=C),
                    )
    nc.finalize()
    return nc


def _get_nc():
    if "nc" not in _CACHE:
        _CACHE["nc"] = _build_module()
    return _CACHE["nc"]


def _get_runner():
    """Cached jitted shard_map executable (run_bass_via_pjrt rebuilds its jit
    closure per call, reloading the executable each time; this caches it)."""
    if "runner" in _CACHE:
        return _CACHE["runner"]
    import jax
    import jax.numpy as jnp
    from jax.experimental.shard_map import shard_map
    from jax.sharding import Mesh, PartitionSpec

    import concourse.mybir as mybir
    from concourse import bass2jax

    nc = _get_nc()
    bass2jax.install_neuronx_cc_hook()

    partition_name = nc.partition_id_tensor.name if nc.partition_id_tensor else None
    in_names, out_names, out_avals = [], [], []
    for alloc in nc.m.functions[0].allocations:
        if not isinstance(alloc, mybir.MemoryLocationSet):
            continue
        name = alloc.memorylocations[0].name
        if alloc.kind == "ExternalInput":
            if name != partition_name:
                in_names.append(name)
        elif alloc.kind == "ExternalOutput":
            out_names.append(name)
            out_avals.append(
                jax.core.ShapedArray(
                    tuple(alloc.tensor_shape), mybir.dt.np(alloc.dtype)
                )
            )
    n_params = len(in_names)
    n_outs = len(out_names)
    all_names = [*in_names, *out_names]
    if partition_name is not None:
        all_names.append(partition_name)

    def _body(*args):
        operands = list(args)
        if partition_name is not None:
            operands.append(bass2jax.partition_id_tensor())
        outs = bass2jax._bass_exec_p.bind(
            *operands,
            out_avals=tuple(out_avals),
            in_names=tuple(all_names),
            out_names=tuple(out_names),
            lowering_input_output_aliases=(),
            sim_require_finite=True,
            sim_require_nnan=True,
            nc=nc,
        )
        return tuple(outs)

    devices = jax.devices()[:N_CORES]
    mesh = Mesh(np.asarray(devices), ("core",))
    sharded = jax.jit(
        shard_map(
            _body,
            mesh=mesh,
            in_specs=(PartitionSpec("core"),) * (n_params + n_outs),
            out_specs=(PartitionSpec("core"),) * n_outs,
            check_rep=False,
        ),
        donate_argnums=tuple(range(n_params, n_params + n_outs)),
        keep_unused=True,
    )
    # Donated output buffers made on-device (no host->device zero transfer).
    zero_makers = [
        jax.jit(
            lambda shape=tuple(a.shape), dtype=a.dtype: jnp.zeros(
                (N_CORES * shape[0], *shape[1:]), dtype
            )
        )
        for a in out_avals
    ]

    def run(updates, mask):
        ins = {"updates": updates, "mask": mask}
        out_arrs = sharded(
            *[ins[name] for name in in_names], *[mk() for mk in zero_makers]
        )
        return np.asarray(out_arrs[out_names.index("out")])

    _CACHE["runner"] = run
    return run


def _run(updates, mask, trace=False):
    updates = np.ascontiguousarray(updates, dtype=np.float32)
    mask = np.ascontiguousarray(mask, dtype=np.int32)

    if not trace:
        return _get_runner()(updates, mask), None

    # Profiling path (test.py): go through the library so NTFF capture works.
    from concourse.bass_utils import run_bass_kernel_spmd

    nc = _get_nc()
    in_maps = [
        {
            "updates": updates[i * BPC : (i + 1) * BPC],
            "mask": mask[i * BPC : (i + 1) * BPC],
        }
        for i in range(N_CORES)
    ]
    res = run_bass_kernel_spmd(
        nc,
        in_maps,
        core_ids=list(range(N_CORES)),
        trace=trace,
    )
    out = np.concatenate([r["out"] for r in res.results], axis=0)
    return out, res


def kernel(**inputs):
    out, _ = _run(inputs["updates"], inputs["mask"])
    return out
